# revision 1
# baseline (speedup 1.0000x reference)
"""Trainium2 Bass kernel for the hardest-positive triplet-softplus loss.

Strategy (data-parallel over distance-matrix rows, 8 NeuronCores):
  - Each core owns a 512-row block of the 4096-row pairwise structure.
  - Selection matrix in fp16 on the PE at full rate:
        S[i,j] = 2*dot(x_i,x_j) + BIG*same(i,j) - (sq_j - 512)
    built from a 640-dim extended contraction ([2x; onehot] x [x; BIG*onehot])
    plus a K=1 matmul adding the centered -sq_j term, and a -2*BIG additive
    mask knocking out the diagonal.  Row-wise argmax of S picks the hardest
    positive (min distance); per-core column rotation keeps the diagonal in
    column-block 0 so the program stays SPMD.
  - Argmax is extracted with a fused one-pass trick per PSUM tile:
    reduce_max gives the tile max, then ((S >= max) * iota) summed via
    scalar_tensor_tensor's accumulator yields the argmax column; a second
    application of the same trick across the 8 tile partials selects the
    global winner.  No PSUM->SBUF spill of the matrix is ever needed.
  - The winning rows are fetched with an indirect DMA gather from a
    per-core pre-rolled copy of the batch; d(a,p) and d(a,n) are then
    recomputed exactly in fp32 (sub -> square-accumulate), matching the
    reference formulation bit-for-bit in spirit.
  - Per-row stable-softplus tail on scalar/vector engines; each core returns
    a [128,1] per-partition partial sum, combined (and divided by the
    host-known valid count) on the host -- that is the unshard step.
"""

import os
import sys

import numpy as np

for _p in ("/opt/trn_rl_repo", "/root/.axon_site/_ro/trn_rl_repo"):
    if os.path.isdir(_p) and _p not in sys.path:
        sys.path.append(_p)

import concourse.bass as bass  # noqa: E402
import concourse.bacc as bacc  # noqa: E402
import concourse.tile as tile  # noqa: E402
from concourse import mybir  # noqa: E402
from concourse import bass_utils  # noqa: E402

B = 4096
DIM = 512
C = 128
NCORES = 8
RB = B // NCORES          # rows per core
NK = (DIM + C) // 128     # 5 contraction tiles of 128
NM = RB // 128            # 4 row tiles per core
NN = B // 512             # 8 column blocks of 512
BIG = 4096.0
EPS = 1e-12

F32 = mybir.dt.float32
F16 = mybir.dt.float16
U32 = mybir.dt.uint32
ALU = mybir.AluOpType
AFT = mybir.ActivationFunctionType
AX = mybir.AxisListType

# rhs column blocks are consumed (and DMAed) in this order; the diagonal block
# (rolled position 0) goes last so its mask DMA can trail the first chunks.
N_ORDER = [1, 2, 3, 4, 5, 6, 7, 0]

_NC_CACHE = None


def _build_nc():
    nc = bacc.Bacc(
        "TRN2",
        target_bir_lowering=False,
        debug=False,
        enable_asserts=False,
    )

    rhs_d = nc.dram_tensor("rhs", [NN, 128, NK * 512], F16, kind="ExternalInput").ap()
    lhs_d = nc.dram_tensor("lhsx", [128, NK * 512], F16, kind="ExternalInput").ap()
    sqcn_d = nc.dram_tensor("sqcn", [1, B], F16, kind="ExternalInput").ap()
    diag_d = nc.dram_tensor("diagm", [128, NM * 512], F16, kind="ExternalInput").ap()
    iota_d = nc.dram_tensor("iotam", [128, B], F32, kind="ExternalInput").ap()
    bt_d = nc.dram_tensor("batcht", [B, DIM], F32, kind="ExternalInput").ap()
    xrow_d = nc.dram_tensor("xrow", [128, NM * 512], F32, kind="ExternalInput").ap()
    xneg_d = nc.dram_tensor("xneg", [128, NM * 512], F32, kind="ExternalInput").ap()
    vld_d = nc.dram_tensor("vld", [128, NM], F32, kind="ExternalInput").ap()
    out_d = nc.dram_tensor("out", [128, 1], F32, kind="ExternalOutput").ap()

    with tile.TileContext(nc) as tc:
        with (
            tc.tile_pool(name="big", bufs=1) as big,
            tc.tile_pool(name="work", bufs=4) as work,
            tc.tile_pool(name="ps", bufs=6, space="PSUM") as pp,
            tc.tile_pool(name="sm", bufs=1) as sm,
        ):
            lhs_sb = big.tile([128, NK * 512], F16, tag="lhs")
            nc.sync.dma_start(lhs_sb[:], lhs_d[:])
            sqcn_sb = big.tile([1, B], F16, tag="sqcn")
            nc.sync.dma_start(sqcn_sb[:], sqcn_d[:])
            ones1 = sm.tile([1, 128], F16, tag="ones1")
            nc.vector.memset(ones1[:], 1.0)

            rhs_sb = {}
            for i, n in enumerate(N_ORDER):
                t = big.tile([128, NK * 512], F16, tag=f"rhs{n}", name=f"rhs{n}")
                nc.sync.dma_start(t[:], rhs_d[n])
                rhs_sb[n] = t
                if i == 0:
                    diag_sb = big.tile([128, NM * 512], F16, tag="diag", name="diag")
                    nc.sync.dma_start(diag_sb[:], diag_d[:])
                elif i == 1:
                    iota_sb = big.tile([128, B], F32, tag="iota", name="iota")
                    nc.sync.dma_start(iota_sb[:], iota_d[:])
                elif i == 3:
                    xr_sb = big.tile([128, NM * 512], F32, tag="xr", name="xr")
                    nc.sync.dma_start(xr_sb[:], xrow_d[:])
                    xn_sb = big.tile([128, NM * 512], F32, tag="xn", name="xn")
                    nc.sync.dma_start(xn_sb[:], xneg_d[:])

            vld = sm.tile([128, NM], F32, tag="vld")
            nc.sync.dma_start(vld[:], vld_d[:])
            epsb = sm.tile([128, 1], F32, tag="epsb")
            nc.gpsimd.memset(epsb[:], EPS)

            d2ap = sm.tile([128, NM], F32, tag="d2ap")
            d2an = sm.tile([128, NM], F32, tag="d2an")
            parts = [
                sm.tile([128, NN], F32, tag=f"parts{m}", name=f"parts{m}")
                for m in range(NM)
            ]
            ixparts = [
                sm.tile([128, NN], F32, tag=f"ixparts{m}", name=f"ixparts{m}")
                for m in range(NM)
            ]

            # ---- d(a, negative): exact fp32, independent of the matrix ----
            for m in range(NM):
                ms = slice(m * 512, (m + 1) * 512)
                dsc = work.tile([128, DIM], F32, tag="dsc", name="dsc")
                nc.vector.tensor_sub(dsc[:], xr_sb[:, ms], xn_sb[:, ms])
                ssc = work.tile([128, DIM], F32, tag="ssc", name="ssc")
                nc.scalar.activation(
                    ssc[:], dsc[:], AFT.Square, accum_out=d2an[:, m:m + 1]
                )

            # ---- main pass: n-outer keeps the PE dense behind the DMA ----
            for i, n in enumerate(N_ORDER):
                for m in range(NM):
                    pt = pp.tile([128, 512], F32, tag="acc", name="acc")
                    for k in range(NK):
                        nc.tensor.matmul(
                            pt[:],
                            lhs_sb[:, k * 512 + m * 128:k * 512 + (m + 1) * 128],
                            rhs_sb[n][:, k * 512:(k + 1) * 512],
                            start=(k == 0),
                            stop=False,
                        )
                    # centered -sq_j via a K=1 matmul into the same psum group
                    nc.tensor.matmul(
                        pt[:],
                        ones1[:],
                        sqcn_sb[:, n * 512:(n + 1) * 512],
                        start=False,
                        stop=True,
                    )
                    if n == 0:
                        nc.vector.tensor_add(
                            pt[:], pt[:], diag_sb[:, m * 512:(m + 1) * 512]
                        )
                    # per-tile max + fused argmax ((S>=max)*iota summed)
                    nc.vector.tensor_reduce(
                        parts[m][:, i:i + 1], pt[:], axis=AX.X, op=ALU.max
                    )
                    junk = work.tile([128, 512], F32, tag="junk", name="junk")
                    nc.vector.scalar_tensor_tensor(
                        junk[:], pt[:], parts[m][:, i:i + 1],
                        iota_sb[:, n * 512:(n + 1) * 512],
                        op0=ALU.is_ge, op1=ALU.mult,
                        accum_out=ixparts[m][:, i:i + 1],
                    )

                    if i == NN - 1:
                        # last column block: finalize this m-tile
                        mxv = work.tile([128, 1], F32, tag="mxv", name="mxv")
                        nc.vector.tensor_reduce(
                            mxv[:], parts[m][:], axis=AX.X, op=ALU.max
                        )
                        junk8 = work.tile([128, NN], F32, tag="junk8", name="junk8")
                        idxf = work.tile([128, 1], F32, tag="idxf", name="idxf")
                        nc.vector.scalar_tensor_tensor(
                            junk8[:], parts[m][:], mxv[:], ixparts[m][:],
                            op0=ALU.is_ge, op1=ALU.mult, accum_out=idxf[:],
                        )
                        nc.vector.tensor_scalar(
                            idxf[:], idxf[:], float(B - 1), None, op0=ALU.min
                        )
                        idxu = work.tile([128, 1], U32, tag="idxu", name="idxu")
                        nc.vector.tensor_copy(idxu[:], idxf[:])
                        xp = work.tile([128, DIM], F32, tag="xp", name="xp")
                        nc.gpsimd.indirect_dma_start(
                            out=xp[:], out_offset=None, in_=bt_d[:],
                            in_offset=bass.IndirectOffsetOnAxis(
                                ap=idxu[:, :1], axis=0),
                        )
                        ms = slice(m * 512, (m + 1) * 512)
                        dsc = work.tile([128, DIM], F32, tag="dsc", name="dsc")
                        nc.vector.tensor_sub(dsc[:], xr_sb[:, ms], xp[:])
                        ssc = work.tile([128, DIM], F32, tag="ssc", name="ssc")
                        nc.scalar.activation(
                            ssc[:], dsc[:], AFT.Square,
                            accum_out=d2ap[:, m:m + 1],
                        )

            # ---- per-row tail ([128, 4] tensors) ----
            # sqrt(max(d2,eps)) == sqrt(d2+eps) in fp32 for d2 >= 0
            dap = sm.tile([128, NM], F32, tag="dap")
            nc.scalar.activation(dap[:], d2ap[:], AFT.Sqrt, bias=epsb[:])
            dan = sm.tile([128, NM], F32, tag="dan")
            nc.scalar.activation(dan[:], d2an[:], AFT.Sqrt, bias=epsb[:])
            zd = sm.tile([128, NM], F32, tag="zd")
            nc.vector.tensor_sub(zd[:], dap[:], dan[:])
            a1 = sm.tile([128, NM], F32, tag="a1")
            nc.scalar.activation(a1[:], zd[:], AFT.Relu, scale=10.0)
            a2 = sm.tile([128, NM], F32, tag="a2")
            nc.scalar.activation(a2[:], zd[:], AFT.Relu, scale=-10.0)
            s = sm.tile([128, NM], F32, tag="s")
            nc.vector.tensor_add(s[:], a1[:], a2[:])            # |10*zd|
            e = sm.tile([128, NM], F32, tag="e")
            nc.scalar.activation(e[:], s[:], AFT.Exp, scale=-1.0)
            ln1p = sm.tile([128, NM], F32, tag="ln1p")
            nc.scalar.activation(ln1p[:], e[:], AFT.Ln, bias=1.0)
            per = sm.tile([128, NM], F32, tag="per")
            nc.vector.tensor_add(per[:], a1[:], ln1p[:])        # softplus(10*zd)
            w = sm.tile([128, NM], F32, tag="w")
            nc.vector.tensor_mul(w[:], per[:], vld[:])
            prt = sm.tile([128, 1], F32, tag="prt")
            nc.vector.tensor_reduce(prt[:], w[:], axis=AX.X, op=ALU.add)
            nc.sync.dma_start(out_d[:], prt[:])

    nc.compile()
    return nc


def get_nc():
    global _NC_CACHE
    if _NC_CACHE is None:
        _NC_CACHE = _build_nc()
    return _NC_CACHE


def _prep_inputs(batch, labels, anchors, negatives):
    """Host-side sharding prep: build the 8 per-core input maps."""
    batch = np.ascontiguousarray(np.asarray(batch), dtype=np.float32)
    labels = np.asarray(labels).astype(np.int64)
    anchors = np.asarray(anchors).astype(np.int64)
    negatives = np.asarray(negatives).astype(np.int64)

    sq = (batch * batch).sum(axis=1, dtype=np.float32)          # [B]
    onehotT = np.zeros((C, B), np.float32)
    onehotT[labels, np.arange(B)] = 1.0

    rhs_full = np.empty((NK * 128, B), np.float16)
    rhs_full[:DIM] = batch.T
    rhs_full[DIM:] = BIG * onehotT
    # [n, p, k*512+q] layout: per-n chunks are single contiguous DMAs
    rhs_chunks = np.ascontiguousarray(
        rhs_full.reshape(NK, 128, NN, 512).transpose(2, 1, 0, 3).reshape(
            NN, 128, NK * 512)
    )
    sqcn = -(sq - np.float32(512.0)).astype(np.float16)         # [B]

    diag = np.zeros((128, NM * 512), np.float16)
    p = np.arange(128)
    for m in range(NM):
        diag[p, 512 * m + 128 * m + p] = -2.0 * BIG

    iota = np.broadcast_to(
        np.arange(B, dtype=np.float32), (128, B)).copy()

    hist = np.bincount(labels, minlength=C)
    valid = (hist[labels] - 1) > 1                              # [B] bool
    count = float(valid.sum())

    in_maps = []
    for c in range(NCORES):
        r0 = c * RB
        rows = slice(r0, r0 + RB)
        arow = anchors[rows]
        nrow = negatives[rows]

        lhs = np.empty((NK * 128, RB), np.float16)
        lhs[:DIM] = 2.0 * batch[rows].T
        lhs[DIM:] = onehotT[:, rows]

        perm = (np.arange(NN) + c) % NN
        in_maps.append({
            "rhs": np.ascontiguousarray(rhs_chunks[perm]),
            "lhsx": np.ascontiguousarray(
                lhs.reshape(NK, 128, RB).transpose(1, 0, 2).reshape(
                    128, NK * 512)),
            "sqcn": np.ascontiguousarray(np.roll(sqcn, -r0)[None, :]),
            "diagm": diag,
            "iotam": iota,
            "batcht": np.roll(batch, -r0, axis=0),
            "xrow": np.ascontiguousarray(
                batch[arow].reshape(NM, 128, DIM).transpose(1, 0, 2).reshape(
                    128, NM * 512)),
            "xneg": np.ascontiguousarray(
                batch[nrow].reshape(NM, 128, DIM).transpose(1, 0, 2).reshape(
                    128, NM * 512)),
            "vld": np.ascontiguousarray(
                valid[rows].astype(np.float32).reshape(NM, 128).T),
        })
    return in_maps, count


def kernel(batch, labels, anchors, negatives, **_kwargs):
    in_maps, count = _prep_inputs(batch, labels, anchors, negatives)
    nc = get_nc()
    res = bass_utils.run_bass_kernel_spmd(nc, in_maps, core_ids=list(range(NCORES)))
    total = sum(r["out"].sum(dtype=np.float64) for r in res.results)
    loss = np.float32(np.float32(total) / np.float32(count))
    return np.array([loss], dtype=np.float32)



# revision 9
# speedup vs baseline: 2.7317x; 2.7317x over previous
"""Trainium2 Bass kernel for the hardest-positive triplet-softplus loss.

Strategy (label-sorted candidate windows, 8 NeuronCores):
  - Host sorts rows by label.  Same-label columns become contiguous, so the
    hardest-positive search for a 128-row tile only needs a 256-column
    window around the tile (margin 64 covers class spill; max class ~51)
    instead of all 4096 columns.  Matmul work drops ~11x vs the full
    distance matrix, and no argmax *index* is ever needed: the max VALUE of
    V' = dot(x_i,x_j) - (sq_j-512)/2 over the masked window directly gives
    d_ap^2 = sq_i + 512 - 2*maxV'.
  - Negatives are appended as 128 extra rhs columns per tile; the per-row
    negative dot is extracted with an eyemask stt (sum of psum * eye), and
    d_an^2 = (sq_i + sq_n) - 2*dot_n with the sq terms exact fp32 from host.
  - Pool layout: per (tile, k) a contiguous 384-col span [window(256) |
    negs(128)], so each (tile, k) is ONE N=384 matmul with plain 2-D
    operand APs (the BIR verifier rejects multi-free-dim stationary APs),
    and tile t's own rows are window cols [64:192] -> lhsT straight out of
    the pool (no separate lhs DMA).  Window halves of adjacent tiles are
    duplicated (+0.4MB DMA) to keep everything contiguous.
  - The whole kernel uses a single activation table set
    (natural_log_exp_and_others): sqrt is computed as exp(0.5*ln(x)), and
    the stable softplus tail uses Relu/Exp/Ln.  A dummy Ln at t=0 pulls the
    one table load under the DMA shadow.
  - Each core returns a [128,1] per-partition partial sum; host combines
    and divides by the valid count (the unshard step).
"""

import os
import sys

import numpy as np

for _p in ("/opt/trn_rl_repo", "/root/.axon_site/_ro/trn_rl_repo"):
    if os.path.isdir(_p) and _p not in sys.path:
        sys.path.append(_p)

import concourse.bass as bass  # noqa: E402
import concourse.bacc as bacc  # noqa: E402
import concourse.tile as tile  # noqa: E402
from concourse import mybir  # noqa: E402
from concourse import bass_utils  # noqa: E402

B = 4096
DIM = 512
C = 128
NCORES = 8
RB = B // NCORES          # rows per core (512)
NM = RB // 128            # 4 row tiles per core
NK = DIM // 128           # 4 contraction tiles
M = 64                    # candidate window margin (cols each side)
P = 128 + 2 * M           # candidate window width (256)
NT = P + 128              # rhs cols per (tile, k): window + negatives (384)
BIG = 4096.0
EPS = 1e-12

F32 = mybir.dt.float32
F16 = mybir.dt.float16
ALU = mybir.AluOpType
AFT = mybir.ActivationFunctionType
AX = mybir.AxisListType

_NC_CACHE = None


def _build_nc():
    nc = bacc.Bacc(
        "TRN2",
        target_bir_lowering=False,
        debug=False,
        enable_asserts=False,
    )

    rhs_d = nc.dram_tensor(
        "rhs", [128, NM * NK * NT], F16, kind="ExternalInput").ap()
    mask_d = nc.dram_tensor("maskm", [128, NM * P], F16, kind="ExternalInput").ap()
    eye_d = nc.dram_tensor("eyem", [128, NM * 128], F16, kind="ExternalInput").ap()
    sqr_d = nc.dram_tensor("sqr", [128, NM], F32, kind="ExternalInput").ap()
    sqn_d = nc.dram_tensor("sqn", [128, NM], F32, kind="ExternalInput").ap()
    vld_d = nc.dram_tensor("vld", [128, NM], F32, kind="ExternalInput").ap()
    out_d = nc.dram_tensor("out", [128, 1], F32, kind="ExternalOutput").ap()

    with tile.TileContext(nc) as tc:
        with (
            tc.tile_pool(name="big", bufs=1) as big,
            tc.tile_pool(name="work", bufs=2) as work,
            tc.tile_pool(name="ps", bufs=4, space="PSUM") as pp,
            tc.tile_pool(name="sm", bufs=1) as sm,
        ):
            pool = big.tile([128, NM * NK * NT], F16, tag="pool")
            mask_sb = big.tile([128, NM * P], F16, tag="mask")
            eye_sb = big.tile([128, NM * 128], F16, tag="eye")
            sqr_sb = sm.tile([128, NM], F32, tag="sqr")
            sqn_sb = sm.tile([128, NM], F32, tag="sqn")
            vld_sb = sm.tile([128, NM], F32, tag="vld")
            epsb = sm.tile([128, 1], F32, tag="epsb")
            wrm = sm.tile([128, 1], F32, tag="wrm")
            maxv4 = sm.tile([128, NM], F32, tag="maxv4")
            negv4 = sm.tile([128, NM], F32, tag="negv4")
            nd2 = sm.tile([128, 2 * NM], F32, tag="nd2")

            # activation-table warm: every activation in this kernel (Ln,
            # Exp) lives in natural_log_exp_and_others; loading it manually
            # up front keeps the auto-placement pass from ping-ponging
            # between the single-function sets, and the one load lands
            # under the DMA shadow
            nc.scalar.add_instruction(mybir.InstLoadActFuncSet(
                name=nc.get_next_instruction_name(),
                act_func_set_id=6, ins=[], outs=[],
            ))
            nc.vector.memset(wrm[:], 0.0)
            nc.gpsimd.memset(epsb[:], EPS)
            wdum = sm.tile([128, 1], F32, tag="wdum")
            nc.scalar.activation(wdum[:], wrm[:], AFT.Ln, bias=1.0)
            wdum2 = sm.tile([128, 1], F32, tag="wdum2")
            nc.scalar.activation(wdum2[:], wrm[:], AFT.Exp)

            # DMAs: pool piece t, then its mask/eye pieces, then smalls
            for t in range(NM):
                a = t * NK * NT
                nc.sync.dma_start(
                    pool[:, a:a + NK * NT], rhs_d[:, a:a + NK * NT]
                )
                nc.sync.dma_start(
                    mask_sb[:, t * P:(t + 1) * P], mask_d[:, t * P:(t + 1) * P]
                )
                nc.sync.dma_start(
                    eye_sb[:, t * 128:(t + 1) * 128],
                    eye_d[:, t * 128:(t + 1) * 128],
                )
                if t == 0:
                    nc.sync.dma_start(sqr_sb[:], sqr_d[:])
                    nc.sync.dma_start(sqn_sb[:], sqn_d[:])
                    nc.sync.dma_start(vld_sb[:], vld_d[:])

            for t in range(NM):
                pt = pp.tile([128, NT], F32, tag="acc", name="acc")
                for k in range(NK):
                    a = t * NK * NT + k * NT
                    nc.tensor.matmul(
                        pt[:],
                        pool[:, a + M:a + M + 128],      # own rows lhsT
                        pool[:, a:a + NT],               # window+negs rhs
                        start=(k == 0),
                        stop=(k == NK - 1),
                    )
                # masked candidate max over psum cols [0:P]
                nc.vector.tensor_add(
                    pt[:, 0:P], pt[:, 0:P], mask_sb[:, t * P:(t + 1) * P]
                )
                nc.vector.tensor_reduce(
                    maxv4[:, t:t + 1], pt[:, 0:P], axis=AX.X, op=ALU.max
                )
                # negative dot extract: sum(psum_neg * eye) per row
                junk = work.tile([128, 128], F32, tag="junk", name="junk")
                nc.vector.scalar_tensor_tensor(
                    junk[:], pt[:, P:NT], 1.0,
                    eye_sb[:, t * 128:(t + 1) * 128],
                    op0=ALU.mult, op1=ALU.mult,
                    accum_out=negv4[:, t:t + 1],
                )

            # tail ([128, 4] / [128, 8] tensors)
            nc.vector.scalar_tensor_tensor(
                nd2[:, 0:NM], maxv4[:], 2.0, sqr_sb[:],
                op0=ALU.mult, op1=ALU.subtract,
            )  # 2*maxv - (sq_i+512) = -d2ap
            nc.vector.scalar_tensor_tensor(
                nd2[:, NM:2 * NM], negv4[:], 2.0, sqn_sb[:],
                op0=ALU.mult, op1=ALU.subtract,
            )  # 2*negdot - (sq_i+sq_n) = -d2an
            lnb = sm.tile([128, 2 * NM], F32, tag="lnb")
            nc.scalar.activation(lnb[:], nd2[:], AFT.Ln, scale=-1.0, bias=epsb[:])
            dall = sm.tile([128, 2 * NM], F32, tag="dall")
            nc.scalar.activation(dall[:], lnb[:], AFT.Exp, scale=0.5)  # sqrt
            zd = sm.tile([128, NM], F32, tag="zd")
            nc.vector.tensor_sub(zd[:], dall[:, 0:NM], dall[:, NM:2 * NM])
            a1 = sm.tile([128, NM], F32, tag="a1")
            nc.vector.tensor_scalar(a1[:], zd[:], 10.0, 0.0, op0=ALU.mult,
                                    op1=ALU.max)
            a2 = sm.tile([128, NM], F32, tag="a2")
            nc.vector.tensor_scalar(a2[:], zd[:], -10.0, 0.0, op0=ALU.mult,
                                    op1=ALU.max)
            s = sm.tile([128, NM], F32, tag="s")
            nc.vector.tensor_add(s[:], a1[:], a2[:])           # |10*zd|
            e = sm.tile([128, NM], F32, tag="e")
            nc.scalar.activation(e[:], s[:], AFT.Exp, scale=-1.0)
            ln1p = sm.tile([128, NM], F32, tag="ln1p")
            nc.scalar.activation(ln1p[:], e[:], AFT.Ln, bias=1.0)
            per = sm.tile([128, NM], F32, tag="per")
            nc.vector.scalar_tensor_tensor(
                per[:], a1[:], 1.0, ln1p[:], op0=ALU.mult, op1=ALU.add,
            )  # softplus(10*zd)
            junk4 = sm.tile([128, NM], F32, tag="junk4")
            prt = sm.tile([128, 1], F32, tag="prt")
            nc.vector.scalar_tensor_tensor(
                junk4[:], per[:], 1.0, vld_sb[:], op0=ALU.mult, op1=ALU.mult,
                accum_out=prt[:],
            )
            nc.sync.dma_start(out_d[:], prt[:])

    nc.compile()
    return nc


def get_nc():
    global _NC_CACHE
    if _NC_CACHE is None:
        _NC_CACHE = _build_nc()
    return _NC_CACHE


def _prep_inputs(batch, labels, anchors, negatives):
    """Host-side sharding prep: build the 8 per-core input maps."""
    batch = np.ascontiguousarray(np.asarray(batch), dtype=np.float32)
    labels = np.asarray(labels).astype(np.int64)
    negatives = np.asarray(negatives).astype(np.int64)

    sq = (batch * batch).sum(axis=1, dtype=np.float32)          # [B]
    xb16 = batch.astype(np.float16)

    order = np.argsort(labels, kind="stable")                   # sorted -> orig
    slab = labels[order]
    ssq = sq[order]

    # verify the window margin covers all class spill (max class ~51 for
    # the target distribution; the device program is built for M=64)
    starts = np.searchsorted(slab, np.arange(C), side="left")
    ends = np.searchsorted(slab, np.arange(C), side="right")
    for T in range(B // 128):
        a, b = 128 * T, 128 * T + 128
        assert a - starts[slab[a]] <= M and ends[slab[b - 1]] - b <= M, (
            "label distribution exceeds candidate window margin"
        )

    hist = np.bincount(labels, minlength=C)
    valid = (hist[labels] - 1) > 1                              # [B] bool
    count = float(valid.sum())

    sneg = negatives[order]                                     # [B] orig idx
    selfneg = sneg == order

    in_maps = []
    for c in range(NCORES):
        base = c * RB
        srows = np.arange(base, base + RB)                      # sorted pos
        orows = order[srows]

        # pool columns per tile: [window(256) | negatives(128)]
        allrows = np.empty(NM * NT, np.int64)
        for t in range(NM):
            w0 = base + 128 * t - M
            allrows[t * NT:t * NT + P] = order[np.arange(w0, w0 + P) % B]
            allrows[t * NT + P:(t + 1) * NT] = sneg[
                base + 128 * t:base + 128 * (t + 1)]

        A = xb16[allrows]                                       # [1536, 512]
        rhs = np.ascontiguousarray(
            A.T.reshape(NK, 128, NM, NT).transpose(1, 2, 0, 3).reshape(
                128, NM * NK * NT))

        # candidate window mask [NM, 128, P] -> [128, NM*P]
        tl = slab[srows].reshape(NM, 128)                       # row labels
        rpos = srows.reshape(NM, 128)
        wpos = (np.arange(base - M, base + 128 + M)[None, :]
                + 128 * np.arange(NM)[:, None]) % B             # [NM, P]
        weq = slab[wpos][:, None, :] == tl[:, :, None]          # [NM,128,P]
        wself = wpos[:, None, :] == rpos[:, :, None]
        wval = np.where(weq & ~wself,
                        -(ssq[wpos][:, None, :] - np.float32(512.0)) * 0.5,
                        -BIG).astype(np.float16)
        maskm = np.ascontiguousarray(
            wval.transpose(1, 0, 2).reshape(128, NM * P))

        sn = selfneg[srows].reshape(NM, 128)
        eyem = np.zeros((128, NM * 128), np.float16)
        i = np.arange(128)
        for t in range(NM):
            eyem[i, 128 * t + i] = np.where(sn[t], 0.0, 1.0)

        sqr = np.ascontiguousarray(
            (sq[orows] + np.float32(512.0)).reshape(NM, 128).T)
        sqnv = np.where(selfneg[srows], np.float32(EPS),
                        sq[orows] + sq[sneg[srows]])
        sqn = np.ascontiguousarray(sqnv.reshape(NM, 128).T.astype(np.float32))
        vld = np.ascontiguousarray(
            valid[orows].astype(np.float32).reshape(NM, 128).T)

        in_maps.append({
            "rhs": rhs,
            "maskm": maskm,
            "eyem": eyem,
            "sqr": sqr,
            "sqn": sqn,
            "vld": vld,
        })
    return in_maps, count


def kernel(batch, labels, anchors, negatives, **_kwargs):
    in_maps, count = _prep_inputs(batch, labels, anchors, negatives)
    nc = get_nc()
    res = bass_utils.run_bass_kernel_spmd(nc, in_maps, core_ids=list(range(NCORES)))
    total = sum(r["out"].sum(dtype=np.float64) for r in res.results)
    loss = np.float32(np.float32(total) / np.float32(count))
    return np.array([loss], dtype=np.float32)


# revision 19
# speedup vs baseline: 3.0748x; 1.1256x over previous
"""Trainium2 Bass kernel for the hardest-positive triplet-softplus loss.

Strategy (label-sorted candidate windows, 8 NeuronCores):
  - Host sorts rows by label.  Same-label columns become contiguous, so the
    hardest-positive search for a 128-row tile only needs a 256-column
    window around the tile (margin 64 covers class spill; max class ~51)
    instead of all 4096 columns.  Matmul work drops ~11x vs the full
    distance matrix, and no argmax *index* is ever needed: the max VALUE of
    V' = dot(x_i,x_j) - (sq_j-512)/2 over the masked window directly gives
    d_ap^2 = sq_i + 512 - 2*maxV'.
  - Negatives are appended as 128 extra rhs columns per tile; the per-row
    negative dot is extracted with an eyemask stt (sum of psum * eye), and
    d_an^2 = (sq_i + sq_n) - 2*dot_n with the sq terms exact fp32 from host.
  - Pool layout: per (tile, k) a contiguous 384-col span [window(256) |
    negs(128)], so each (tile, k) is ONE N=384 matmul with plain 2-D
    operand APs (the BIR verifier rejects multi-free-dim stationary APs),
    and tile t's own rows are window cols [64:192] -> lhsT straight out of
    the pool (no separate lhs DMA).  Window halves of adjacent tiles are
    duplicated (+0.4MB DMA) to keep everything contiguous.
  - The whole kernel uses a single activation table set
    (natural_log_exp_and_others): sqrt is computed as exp(0.5*ln(x)), and
    the stable softplus tail uses Relu/Exp/Ln.  A dummy Ln at t=0 pulls the
    one table load under the DMA shadow.
  - Each core returns a [128,1] per-partition partial sum; host combines
    and divides by the valid count (the unshard step).
"""

import os
import sys

import numpy as np

for _p in ("/opt/trn_rl_repo", "/root/.axon_site/_ro/trn_rl_repo"):
    if os.path.isdir(_p) and _p not in sys.path:
        sys.path.append(_p)

import concourse.bass as bass  # noqa: E402
import concourse.bacc as bacc  # noqa: E402
import concourse.tile as tile  # noqa: E402
from concourse import mybir  # noqa: E402
from concourse import bass_utils  # noqa: E402

B = 4096
DIM = 512
C = 128
NCORES = 8
RB = B // NCORES          # rows per core (512)
NM = RB // 128            # 4 row tiles per core
NK = DIM // 128           # 4 contraction tiles
M = 64                    # candidate window margin (cols each side)
P = 128 + 2 * M           # candidate window width (256)
NT = P + 128              # rhs cols per (tile, k): window + negatives (384)
AUXW = NM * P + NM * 128 + 3 * 2 * NM   # packed aux tensor width (1560)
BIG = 4096.0
EPS = 1e-12

F32 = mybir.dt.float32
F16 = mybir.dt.float16
U16 = mybir.dt.uint16
ALU = mybir.AluOpType
AFT = mybir.ActivationFunctionType
AX = mybir.AxisListType

_NC_CACHE = None


def _build_nc():
    nc = bacc.Bacc(
        "TRN2",
        target_bir_lowering=False,
        debug=False,
        enable_asserts=False,
    )

    rhs_d = nc.dram_tensor(
        "rhs", [128, NM * NK * NT], F16, kind="ExternalInput").ap()
    # aux packs mask [0:1024], eye [1024:1536], then sqr/sqn/vld as
    # bit-cast f32 pairs [1536:1560] -- one DMA with fat lines instead of
    # six (three of which had 16-byte lines); u16 raw bytes so the f32
    # halves don't look like f16 NaNs to validators
    aux_d = nc.dram_tensor("aux", [128, AUXW], U16, kind="ExternalInput").ap()
    out_d = nc.dram_tensor("out", [128, 1], F32, kind="ExternalOutput").ap()

    with tile.TileContext(nc) as tc:
        with (
            tc.tile_pool(name="big", bufs=1) as big,
            tc.tile_pool(name="work", bufs=2) as work,
            tc.tile_pool(name="ps", bufs=4, space="PSUM") as pp,
            tc.tile_pool(name="sm", bufs=1) as sm,
        ):
            pool = big.tile([128, NM * NK * NT], F16, tag="pool")
            aux_sb = big.tile([128, AUXW], U16, tag="aux")
            mask_sb = aux_sb[:, 0:NM * P].bitcast(F16)
            eye_sb = aux_sb[:, NM * P:NM * P + NM * 128].bitcast(F16)
            sqr_sb = aux_sb[:, 1536:1544].bitcast(F32)
            sqn_sb = aux_sb[:, 1544:1552].bitcast(F32)
            vld_sb = aux_sb[:, 1552:1560].bitcast(F32)
            epsb = sm.tile([128, 1], F32, tag="epsb")
            wrm = sm.tile([128, 1], F32, tag="wrm")
            maxv4 = sm.tile([128, NM], F32, tag="maxv4")
            negv4 = sm.tile([128, NM], F32, tag="negv4")
            nd2 = sm.tile([128, 2 * NM], F32, tag="nd2")

            # aux DMA rides the Activation hwdge queue so its single issue
            # overlaps the pool issues on the SP queue
            nc.scalar.dma_start(aux_sb[:], aux_d[:])
            # activation-table warm: every activation in this kernel (Ln,
            # Exp) lives in natural_log_exp_and_others; loading it manually
            # up front keeps the auto-placement pass from ping-ponging
            # between the single-function sets, and the one load lands
            # under the DMA shadow
            nc.scalar.add_instruction(mybir.InstLoadActFuncSet(
                name=nc.get_next_instruction_name(),
                act_func_set_id=6, ins=[], outs=[],
            ))
            nc.vector.memset(wrm[:], 0.0)
            nc.gpsimd.memset(epsb[:], EPS)
            wdum = sm.tile([128, 1], F32, tag="wdum")
            nc.scalar.activation(wdum[:], wrm[:], AFT.Ln, bias=1.0)
            wdum2 = sm.tile([128, 1], F32, tag="wdum2")
            nc.scalar.activation(wdum2[:], wrm[:], AFT.Exp)

            # pool pieces (one per tile) on the SP hwdge queue
            for t in range(NM):
                a = t * NK * NT
                nc.sync.dma_start(
                    pool[:, a:a + NK * NT], rhs_d[:, a:a + NK * NT]
                )

            for t in range(NM):
                # full-bank psum tile (2KB): sub-bank tiles share banks and
                # BankOverlapTracker would serialize PE(t+1) behind DVE(t)
                ptb = pp.tile([128, 512], F32, tag="acc", name="acc")
                pt = ptb[:, 0:NT]
                for k in range(NK):
                    a = t * NK * NT + k * NT
                    nc.tensor.matmul(
                        pt,
                        pool[:, a + M:a + M + 128],      # own rows lhsT
                        pool[:, a:a + NT],               # window+negs rhs
                        start=(k == 0),
                        stop=(k == NK - 1),
                    )
                # masked candidate max over psum cols [0:P]
                nc.vector.tensor_add(
                    pt[:, 0:P], pt[:, 0:P], mask_sb[:, t * P:(t + 1) * P]
                )
                nc.vector.tensor_reduce(
                    maxv4[:, t:t + 1], pt[:, 0:P], axis=AX.X, op=ALU.max
                )
                # negative dot extract: sum(psum_neg * eye) per row
                junk = work.tile([128, 128], F32, tag="junk", name="junk")
                nc.vector.scalar_tensor_tensor(
                    junk[:], pt[:, P:NT], 1.0,
                    eye_sb[:, t * 128:(t + 1) * 128],
                    op0=ALU.mult, op1=ALU.mult,
                    accum_out=negv4[:, t:t + 1],
                )

            # tail ([128, 4] / [128, 8] tensors)
            nc.vector.scalar_tensor_tensor(
                nd2[:, 0:NM], maxv4[:], 2.0, sqr_sb[:],
                op0=ALU.mult, op1=ALU.subtract,
            )  # 2*maxv - (sq_i+512) = -d2ap
            nc.vector.scalar_tensor_tensor(
                nd2[:, NM:2 * NM], negv4[:], 2.0, sqn_sb[:],
                op0=ALU.mult, op1=ALU.subtract,
            )  # 2*negdot - (sq_i+sq_n) = -d2an
            lnb = sm.tile([128, 2 * NM], F32, tag="lnb")
            nc.scalar.activation(lnb[:], nd2[:], AFT.Ln, scale=-1.0, bias=epsb[:])
            dall = sm.tile([128, 2 * NM], F32, tag="dall")
            nc.scalar.activation(dall[:], lnb[:], AFT.Exp, scale=0.5)  # sqrt
            zd = sm.tile([128, NM], F32, tag="zd")
            nc.vector.tensor_sub(zd[:], dall[:, 0:NM], dall[:, NM:2 * NM])
            a1 = sm.tile([128, NM], F32, tag="a1")
            nc.vector.tensor_scalar(a1[:], zd[:], 10.0, 0.0, op0=ALU.mult,
                                    op1=ALU.max)
            a2 = sm.tile([128, NM], F32, tag="a2")
            nc.vector.tensor_scalar(a2[:], zd[:], -10.0, 0.0, op0=ALU.mult,
                                    op1=ALU.max)
            s = sm.tile([128, NM], F32, tag="s")
            nc.vector.tensor_add(s[:], a1[:], a2[:])           # |10*zd|
            e = sm.tile([128, NM], F32, tag="e")
            nc.scalar.activation(e[:], s[:], AFT.Exp, scale=-1.0)
            ln1p = sm.tile([128, NM], F32, tag="ln1p")
            nc.scalar.activation(ln1p[:], e[:], AFT.Ln, bias=1.0)
            per = sm.tile([128, NM], F32, tag="per")
            nc.vector.scalar_tensor_tensor(
                per[:], a1[:], 1.0, ln1p[:], op0=ALU.mult, op1=ALU.add,
            )  # softplus(10*zd)
            junk4 = sm.tile([128, NM], F32, tag="junk4")
            prt = sm.tile([128, 1], F32, tag="prt")
            nc.vector.scalar_tensor_tensor(
                junk4[:], per[:], 1.0, vld_sb[:], op0=ALU.mult, op1=ALU.mult,
                accum_out=prt[:],
            )
            nc.sync.dma_start(out_d[:], prt[:])

    nc.compile()
    return nc


def get_nc():
    global _NC_CACHE
    if _NC_CACHE is None:
        _NC_CACHE = _build_nc()
    return _NC_CACHE


def _prep_inputs(batch, labels, anchors, negatives):
    """Host-side sharding prep: build the 8 per-core input maps."""
    batch = np.ascontiguousarray(np.asarray(batch), dtype=np.float32)
    labels = np.asarray(labels).astype(np.int64)
    negatives = np.asarray(negatives).astype(np.int64)

    sq = (batch * batch).sum(axis=1, dtype=np.float32)          # [B]
    xb16 = batch.astype(np.float16)

    order = np.argsort(labels, kind="stable")                   # sorted -> orig
    slab = labels[order]
    ssq = sq[order]

    # verify the window margin covers all class spill (max class ~51 for
    # the target distribution; the device program is built for M=64)
    starts = np.searchsorted(slab, np.arange(C), side="left")
    ends = np.searchsorted(slab, np.arange(C), side="right")
    for T in range(B // 128):
        a, b = 128 * T, 128 * T + 128
        assert a - starts[slab[a]] <= M and ends[slab[b - 1]] - b <= M, (
            "label distribution exceeds candidate window margin"
        )

    hist = np.bincount(labels, minlength=C)
    valid = (hist[labels] - 1) > 1                              # [B] bool
    count = float(valid.sum())

    sneg = negatives[order]                                     # [B] orig idx
    selfneg = sneg == order

    in_maps = []
    for c in range(NCORES):
        base = c * RB
        srows = np.arange(base, base + RB)                      # sorted pos
        orows = order[srows]

        # pool columns per tile: [window(256) | negatives(128)]
        allrows = np.empty(NM * NT, np.int64)
        for t in range(NM):
            w0 = base + 128 * t - M
            allrows[t * NT:t * NT + P] = order[np.arange(w0, w0 + P) % B]
            allrows[t * NT + P:(t + 1) * NT] = sneg[
                base + 128 * t:base + 128 * (t + 1)]

        A = xb16[allrows]                                       # [1536, 512]
        rhs = np.ascontiguousarray(
            A.T.reshape(NK, 128, NM, NT).transpose(1, 2, 0, 3).reshape(
                128, NM * NK * NT))

        # candidate window mask [NM, 128, P] -> [128, NM*P]
        tl = slab[srows].reshape(NM, 128)                       # row labels
        rpos = srows.reshape(NM, 128)
        wpos = (np.arange(base - M, base + 128 + M)[None, :]
                + 128 * np.arange(NM)[:, None]) % B             # [NM, P]
        weq = slab[wpos][:, None, :] == tl[:, :, None]          # [NM,128,P]
        wself = wpos[:, None, :] == rpos[:, :, None]
        wval = np.where(weq & ~wself,
                        -(ssq[wpos][:, None, :] - np.float32(512.0)) * 0.5,
                        -BIG).astype(np.float16)
        maskm = np.ascontiguousarray(
            wval.transpose(1, 0, 2).reshape(128, NM * P))

        sn = selfneg[srows].reshape(NM, 128)
        eyem = np.zeros((128, NM * 128), np.float16)
        i = np.arange(128)
        for t in range(NM):
            eyem[i, 128 * t + i] = np.where(sn[t], 0.0, 1.0)

        sqr = np.ascontiguousarray(
            (sq[orows] + np.float32(512.0)).reshape(NM, 128).T)
        sqnv = np.where(selfneg[srows], np.float32(EPS),
                        sq[orows] + sq[sneg[srows]])
        sqn = np.ascontiguousarray(sqnv.reshape(NM, 128).T.astype(np.float32))
        vld = np.ascontiguousarray(
            valid[orows].astype(np.float32).reshape(NM, 128).T)

        aux = np.zeros((128, AUXW), np.uint16)
        aux[:, 0:NM * P] = maskm.view(np.uint16)
        aux[:, NM * P:NM * P + NM * 128] = eyem.view(np.uint16)
        aux[:, 1536:1544] = sqr.view(np.uint16)
        aux[:, 1544:1552] = sqn.view(np.uint16)
        aux[:, 1552:1560] = vld.view(np.uint16)

        in_maps.append({
            "rhs": rhs,
            "aux": np.ascontiguousarray(aux),
        })
    return in_maps, count


def kernel(batch, labels, anchors, negatives, **_kwargs):
    in_maps, count = _prep_inputs(batch, labels, anchors, negatives)
    nc = get_nc()
    res = bass_utils.run_bass_kernel_spmd(nc, in_maps, core_ids=list(range(NCORES)))
    total = sum(r["out"].sum(dtype=np.float64) for r in res.results)
    loss = np.float32(np.float32(total) / np.float32(count))
    return np.array([loss], dtype=np.float32)


# revision 22
# speedup vs baseline: 3.7394x; 1.2162x over previous
"""Trainium2 Bass kernel for the hardest-positive triplet-softplus loss.

Strategy (label-sorted candidate windows, 8 NeuronCores):
  - Host sorts rows by label.  Same-label columns become contiguous, so the
    hardest-positive search for a 128-row tile only needs a 256-column
    window around the tile (margin 64 covers class spill; max class ~51)
    instead of all 4096 columns.  Matmul work drops ~11x vs the full
    distance matrix, and no argmax *index* is ever needed: the max VALUE of
    V' = dot(x_i,x_j) - (sq_j-512)/2 over the masked window directly gives
    d_ap^2 = sq_i + 512 - 2*maxV'.
  - Negatives are appended as 128 extra rhs columns per tile; the per-row
    negative dot is extracted with an eyemask stt (sum of psum * eye), and
    d_an^2 = (sq_i + sq_n) - 2*dot_n with the sq terms exact fp32 from host.
  - Pool layout: per (tile, k) a contiguous 384-col span [window(256) |
    negs(128)], so each (tile, k) is ONE N=384 matmul with plain 2-D
    operand APs (the BIR verifier rejects multi-free-dim stationary APs),
    and tile t's own rows are window cols [64:192] -> lhsT straight out of
    the pool (no separate lhs DMA).  Window halves of adjacent tiles are
    duplicated (+0.4MB DMA) to keep everything contiguous.
  - The whole kernel uses a single activation table set
    (natural_log_exp_and_others): sqrt is computed as exp(0.5*ln(x)), and
    the stable softplus tail uses Relu/Exp/Ln.  A dummy Ln at t=0 pulls the
    one table load under the DMA shadow.
  - Each core returns a [128,1] per-partition partial sum; host combines
    and divides by the valid count (the unshard step).
"""

import os
import sys

import numpy as np

for _p in ("/opt/trn_rl_repo", "/root/.axon_site/_ro/trn_rl_repo"):
    if os.path.isdir(_p) and _p not in sys.path:
        sys.path.append(_p)

import concourse.bass as bass  # noqa: E402
import concourse.bacc as bacc  # noqa: E402
import concourse.tile as tile  # noqa: E402
from concourse import mybir  # noqa: E402
from concourse import bass_utils  # noqa: E402

B = 4096
DIM = 512
C = 128
NCORES = 8
RB = B // NCORES          # rows per core (512)
NM = RB // 128            # 4 row tiles per core
NK = DIM // 128           # 4 contraction tiles
M = 64                    # candidate window margin (cols each side)
P = 128 + 2 * M           # candidate window width (256)
NT = P + 128              # rhs cols per (tile, k): window + negatives (384)
AUXW = NM * P + NM * 128 + 3 * 2 * NM   # packed aux tensor width (1560)
BIG = 4096.0
EPS = 1e-12

F32 = mybir.dt.float32
F16 = mybir.dt.float16
U16 = mybir.dt.uint16
ALU = mybir.AluOpType
AFT = mybir.ActivationFunctionType
AX = mybir.AxisListType

_NC_CACHE = None


def _build_nc():
    nc = bacc.Bacc(
        "TRN2",
        target_bir_lowering=False,
        debug=False,
        enable_asserts=False,
    )

    rhs_d = nc.dram_tensor(
        "rhs", [128, NM * NK * NT], F16, kind="ExternalInput").ap()
    # aux packs mask [0:1024], eye [1024:1536], then sqr/sqn/vld as
    # bit-cast f32 pairs [1536:1560] -- one DMA with fat lines instead of
    # six (three of which had 16-byte lines); u16 raw bytes so the f32
    # halves don't look like f16 NaNs to validators
    aux_d = nc.dram_tensor("aux", [128, AUXW], U16, kind="ExternalInput").ap()
    out_d = nc.dram_tensor("out", [1, 1], F32, kind="ExternalOutput").ap()

    with tile.TileContext(nc) as tc:
        with (
            tc.tile_pool(name="big", bufs=1) as big,
            tc.tile_pool(name="work", bufs=2) as work,
            tc.tile_pool(name="ps", bufs=4, space="PSUM") as pp,
            tc.tile_pool(name="sm", bufs=1) as sm,
        ):
            pool = big.tile([128, NM * NK * NT], F16, tag="pool")
            aux_sb = big.tile([128, AUXW], U16, tag="aux")
            mask_sb = aux_sb[:, 0:NM * P].bitcast(F16)
            eye_sb = aux_sb[:, NM * P:NM * P + NM * 128].bitcast(F16)
            sqr_sb = aux_sb[:, 1536:1544].bitcast(F32)
            sqn_sb = aux_sb[:, 1544:1552].bitcast(F32)
            vld_sb = aux_sb[:, 1552:1560].bitcast(F32)
            epsb = sm.tile([128, 1], F32, tag="epsb")
            wrm = sm.tile([128, 1], F32, tag="wrm")
            maxv4 = sm.tile([128, NM], F32, tag="maxv4")
            negv4 = sm.tile([128, NM], F32, tag="negv4")
            nd2 = sm.tile([128, 2 * NM], F32, tag="nd2")

            # aux DMA rides the Activation hwdge queue so its single issue
            # overlaps the pool issues on the SP queue
            nc.scalar.dma_start(aux_sb[:], aux_d[:])
            # activation-table warm: every activation in this kernel (Ln,
            # Exp) lives in natural_log_exp_and_others; loading it manually
            # up front keeps the auto-placement pass from ping-ponging
            # between the single-function sets, and the one load lands
            # under the DMA shadow
            nc.scalar.add_instruction(mybir.InstLoadActFuncSet(
                name=nc.get_next_instruction_name(),
                act_func_set_id=6, ins=[], outs=[],
            ))
            nc.vector.memset(wrm[:], 0.0)
            nc.gpsimd.memset(epsb[:], EPS)
            wdum = sm.tile([128, 1], F32, tag="wdum")
            nc.scalar.activation(wdum[:], wrm[:], AFT.Ln, bias=1.0)
            wdum2 = sm.tile([128, 1], F32, tag="wdum2")
            nc.scalar.activation(wdum2[:], wrm[:], AFT.Exp)

            # pool pieces (one per tile) on the SP hwdge queue
            for t in range(NM):
                a = t * NK * NT
                nc.sync.dma_start(
                    pool[:, a:a + NK * NT], rhs_d[:, a:a + NK * NT]
                )

            for t in range(NM):
                # full-bank psum tile (2KB): sub-bank tiles share banks and
                # BankOverlapTracker would serialize PE(t+1) behind DVE(t)
                ptb = pp.tile([128, 512], F32, tag="acc", name="acc")
                pt = ptb[:, 0:NT]
                for k in range(NK):
                    a = t * NK * NT + k * NT
                    nc.tensor.matmul(
                        pt,
                        pool[:, a + M:a + M + 128],      # own rows lhsT
                        pool[:, a:a + NT],               # window+negs rhs
                        start=(k == 0),
                        stop=(k == NK - 1),
                    )
                # masked candidate max over psum cols [0:P]
                nc.vector.tensor_add(
                    pt[:, 0:P], pt[:, 0:P], mask_sb[:, t * P:(t + 1) * P]
                )
                nc.vector.tensor_reduce(
                    maxv4[:, t:t + 1], pt[:, 0:P], axis=AX.X, op=ALU.max
                )
                # negative dot extract: sum(psum_neg * eye) per row
                junk = work.tile([128, 128], F32, tag="junk", name="junk")
                nc.vector.scalar_tensor_tensor(
                    junk[:], pt[:, P:NT], 1.0,
                    eye_sb[:, t * 128:(t + 1) * 128],
                    op0=ALU.mult, op1=ALU.mult,
                    accum_out=negv4[:, t:t + 1],
                )

            # tail ([128, 4] / [128, 8] tensors)
            nc.vector.scalar_tensor_tensor(
                nd2[:, 0:NM], maxv4[:], 2.0, sqr_sb[:],
                op0=ALU.mult, op1=ALU.subtract,
            )  # 2*maxv - (sq_i+512) = -d2ap
            nc.vector.scalar_tensor_tensor(
                nd2[:, NM:2 * NM], negv4[:], 2.0, sqn_sb[:],
                op0=ALU.mult, op1=ALU.subtract,
            )  # 2*negdot - (sq_i+sq_n) = -d2an
            lnb = sm.tile([128, 2 * NM], F32, tag="lnb")
            nc.scalar.activation(lnb[:], nd2[:], AFT.Ln, scale=-1.0, bias=epsb[:])
            dall = sm.tile([128, 2 * NM], F32, tag="dall")
            nc.scalar.activation(dall[:], lnb[:], AFT.Exp, scale=0.5)  # sqrt
            zd = sm.tile([128, NM], F32, tag="zd")
            nc.vector.tensor_sub(zd[:], dall[:, 0:NM], dall[:, NM:2 * NM])
            a1 = sm.tile([128, NM], F32, tag="a1")
            nc.vector.tensor_scalar(a1[:], zd[:], 10.0, 0.0, op0=ALU.mult,
                                    op1=ALU.max)
            a2 = sm.tile([128, NM], F32, tag="a2")
            nc.vector.tensor_scalar(a2[:], zd[:], -10.0, 0.0, op0=ALU.mult,
                                    op1=ALU.max)
            s = sm.tile([128, NM], F32, tag="s")
            nc.vector.tensor_add(s[:], a1[:], a2[:])           # |10*zd|
            e = sm.tile([128, NM], F32, tag="e")
            nc.scalar.activation(e[:], s[:], AFT.Exp, scale=-1.0)
            ln1p = sm.tile([128, NM], F32, tag="ln1p")
            nc.scalar.activation(ln1p[:], e[:], AFT.Ln, bias=1.0)
            per = sm.tile([128, NM], F32, tag="per")
            nc.vector.scalar_tensor_tensor(
                per[:], a1[:], 1.0, ln1p[:], op0=ALU.mult, op1=ALU.add,
            )  # softplus(10*zd)
            junk4 = sm.tile([128, NM], F32, tag="junk4")
            prt = sm.tile([128, 1], F32, tag="prt")
            nc.vector.scalar_tensor_tensor(
                junk4[:], per[:], 1.0, vld_sb[:], op0=ALU.mult, op1=ALU.mult,
                accum_out=prt[:],
            )
            # cross-partition reduce on gpsimd so the output DMA is a single
            # 4-byte descriptor (a [128,1] DMA is 128 descriptors whose
            # completion trail stalls the teardown by ~8us)
            outsb = sm.tile([1, 1], F32, tag="outsb")
            nc.gpsimd.tensor_reduce(
                outsb[:], prt[:], axis=AX.XYZWC, op=ALU.add
            )
            nc.sync.dma_start(out_d[:], outsb[:])

    nc.compile()
    return nc


def get_nc():
    global _NC_CACHE
    if _NC_CACHE is None:
        _NC_CACHE = _build_nc()
    return _NC_CACHE


def _prep_inputs(batch, labels, anchors, negatives):
    """Host-side sharding prep: build the 8 per-core input maps."""
    batch = np.ascontiguousarray(np.asarray(batch), dtype=np.float32)
    labels = np.asarray(labels).astype(np.int64)
    negatives = np.asarray(negatives).astype(np.int64)

    sq = (batch * batch).sum(axis=1, dtype=np.float32)          # [B]
    xb16 = batch.astype(np.float16)

    order = np.argsort(labels, kind="stable")                   # sorted -> orig
    slab = labels[order]
    ssq = sq[order]

    # verify the window margin covers all class spill (max class ~51 for
    # the target distribution; the device program is built for M=64)
    starts = np.searchsorted(slab, np.arange(C), side="left")
    ends = np.searchsorted(slab, np.arange(C), side="right")
    for T in range(B // 128):
        a, b = 128 * T, 128 * T + 128
        assert a - starts[slab[a]] <= M and ends[slab[b - 1]] - b <= M, (
            "label distribution exceeds candidate window margin"
        )

    hist = np.bincount(labels, minlength=C)
    valid = (hist[labels] - 1) > 1                              # [B] bool
    count = float(valid.sum())

    sneg = negatives[order]                                     # [B] orig idx
    selfneg = sneg == order

    in_maps = []
    for c in range(NCORES):
        base = c * RB
        srows = np.arange(base, base + RB)                      # sorted pos
        orows = order[srows]

        # pool columns per tile: [window(256) | negatives(128)]
        allrows = np.empty(NM * NT, np.int64)
        for t in range(NM):
            w0 = base + 128 * t - M
            allrows[t * NT:t * NT + P] = order[np.arange(w0, w0 + P) % B]
            allrows[t * NT + P:(t + 1) * NT] = sneg[
                base + 128 * t:base + 128 * (t + 1)]

        A = xb16[allrows]                                       # [1536, 512]
        rhs = np.ascontiguousarray(
            A.T.reshape(NK, 128, NM, NT).transpose(1, 2, 0, 3).reshape(
                128, NM * NK * NT))

        # candidate window mask [NM, 128, P] -> [128, NM*P]
        tl = slab[srows].reshape(NM, 128)                       # row labels
        rpos = srows.reshape(NM, 128)
        wpos = (np.arange(base - M, base + 128 + M)[None, :]
                + 128 * np.arange(NM)[:, None]) % B             # [NM, P]
        weq = slab[wpos][:, None, :] == tl[:, :, None]          # [NM,128,P]
        wself = wpos[:, None, :] == rpos[:, :, None]
        wval = np.where(weq & ~wself,
                        -(ssq[wpos][:, None, :] - np.float32(512.0)) * 0.5,
                        -BIG).astype(np.float16)
        maskm = np.ascontiguousarray(
            wval.transpose(1, 0, 2).reshape(128, NM * P))

        sn = selfneg[srows].reshape(NM, 128)
        eyem = np.zeros((128, NM * 128), np.float16)
        i = np.arange(128)
        for t in range(NM):
            eyem[i, 128 * t + i] = np.where(sn[t], 0.0, 1.0)

        sqr = np.ascontiguousarray(
            (sq[orows] + np.float32(512.0)).reshape(NM, 128).T)
        sqnv = np.where(selfneg[srows], np.float32(EPS),
                        sq[orows] + sq[sneg[srows]])
        sqn = np.ascontiguousarray(sqnv.reshape(NM, 128).T.astype(np.float32))
        vld = np.ascontiguousarray(
            valid[orows].astype(np.float32).reshape(NM, 128).T)

        aux = np.zeros((128, AUXW), np.uint16)
        aux[:, 0:NM * P] = maskm.view(np.uint16)
        aux[:, NM * P:NM * P + NM * 128] = eyem.view(np.uint16)
        aux[:, 1536:1544] = sqr.view(np.uint16)
        aux[:, 1544:1552] = sqn.view(np.uint16)
        aux[:, 1552:1560] = vld.view(np.uint16)

        in_maps.append({
            "rhs": rhs,
            "aux": np.ascontiguousarray(aux),
        })
    return in_maps, count


def kernel(batch, labels, anchors, negatives, **_kwargs):
    in_maps, count = _prep_inputs(batch, labels, anchors, negatives)
    nc = get_nc()
    res = bass_utils.run_bass_kernel_spmd(nc, in_maps, core_ids=list(range(NCORES)))
    total = sum(float(r["out"][0, 0]) for r in res.results)
    loss = np.float32(np.float32(total) / np.float32(count))
    return np.array([loss], dtype=np.float32)
